# revision 37
# baseline (speedup 1.0000x reference)
"""Trainium2 kernel for per-task MLP routing (MoE-style dictionary model).

Computation (reference):
    l1 = l1_emb[task_ids] -> [B, 256, 64]; l2 = l2_emb[task_ids] -> [B, 64, 64]
    l3 = l3_emb[task_ids] -> [B, 64]
    h1 = gelu(x @ l1); h2 = gelu(h1 @ l2); out = sigmoid(sum(h2*l3))  [B, 1]

Strategy: expert-parallel over tasks. Tasks t in [128*c, 128*(c+1)) live on
core c. The host routes samples to cores by task id, groups each task's
samples into fixed-capacity slots (CAP rows), and pre-gathers/pre-transposes
the per-slot weights so every device-side DMA is large and contiguous.
On-device, each slot is a tiny weight-stationary matmul chain kept entirely
in PSUM/SBUF; slots are processed two-at-a-time in disjoint halves of the
PE array (column/quadrant tiling). GELU runs on the ACT engine (exact-erf
table); the single final Sigmoid pass avoids per-group ACT table swaps.
"""

import numpy as np

F = 256          # features
H = 64           # hidden
NT = 1024        # num tasks
NCORES = 8
TPC = NT // NCORES   # tasks per core
CAP = 16             # sample rows per slot
GP = 11              # slot-pairs per group
GCOLS = GP * CAP     # max psum columns per group

_PROGRAM_CACHE = {}
USE_BF16 = True      # halves HBM traffic; PSUM still accumulates fp32
LAST_IN_MAPS = None  # stashed for test.py's timing harness
LAST_NPAIRS = None


def _build_program(n_pairs, passes=1):
    from contextlib import ExitStack

    import concourse.bacc as bacc
    import concourse.tile as tile
    from concourse import mybir

    f32 = mybir.dt.float32
    fwk = mybir.dt.bfloat16 if USE_BF16 else mybir.dt.float32
    S = 2 * n_pairs
    COLS = n_pairs * CAP
    NG = (n_pairs + GP - 1) // GP

    nc = bacc.Bacc("TRN2", target_bir_lowering=False)
    xs_d = nc.declare_dram_parameter("xs", [2, 128, S * CAP], fwk, False)
    w1_d = nc.declare_dram_parameter("w1", [2, 128, n_pairs * 128], fwk, False)
    w2_d = nc.declare_dram_parameter("w2", [128, n_pairs * H], fwk, False)
    w3_d = nc.declare_dram_parameter("w3e", [128, COLS], fwk, False)
    on_d = nc.declare_dram_parameter("ones2", [128, 2 + GCOLS], fwk, False)
    out_d = nc.declare_dram_parameter("out", [2, COLS], f32, True)

    GELU = mybir.ActivationFunctionType.Gelu
    SIGM = mybir.ActivationFunctionType.Sigmoid

    with ExitStack() as ctx:
        tc = ctx.enter_context(tile.TileContext(nc))
        singles = ctx.enter_context(tc.tile_pool(name="singles", bufs=1))
        wpool = ctx.enter_context(tc.tile_pool(name="wpool", bufs=6))
        hpool = ctx.enter_context(tc.tile_pool(name="hpool", bufs=4))
        # One psum pool per tile tag: a shared pool recycles banks across
        # tags in allocation order, which creates cross-group bank WAW deps
        # that defeat the PE anchor below.
        p1pool = ctx.enter_context(tc.tile_pool(name="psum1", bufs=3, space="PSUM"))
        p2pool = ctx.enter_context(tc.tile_pool(name="psum2", bufs=3, space="PSUM"))
        p3pool = ctx.enter_context(tc.tile_pool(name="psum3", bufs=2, space="PSUM"))

        # Whole-core residents: routed activations (transposed), expanded l3,
        # the partition-half indicator columns, and the logit accumulator.
        xs_sb = []
        for k in range(2):
            t = singles.tile([128, S * CAP], fwk, tag=f"xs{k}")
            nc.sync.dma_start(out=t, in_=xs_d[k])
            xs_sb.append(t)
        w3_sb = singles.tile([128, COLS], fwk, tag="w3e")
        nc.sync.dma_start(out=w3_sb, in_=w3_d[:])
        ones_sb = singles.tile([128, 2 + GCOLS], fwk, tag="ones2")
        nc.sync.dma_start(out=ones_sb, in_=on_d[:])
        logits_sb = singles.tile([2, COLS], f32, tag="logits")
        outsb = singles.tile([2, COLS], f32, tag="outsb")
        for g in range(NG * passes):
            g = g % NG
            p0 = g * GP
            c0 = p0 * CAP                 # each pair contributes CAP columns
            GPg = min(GP, n_pairs - p0)   # last group may be ragged
            GC = GPg * CAP                # psum cols this group

            w1_sb = []
            for k in range(2):
                t = wpool.tile([128, GPg * 128], fwk, tag=f"w1_{k}")
                nc.sync.dma_start(out=t, in_=w1_d[k, :, p0 * 128 : (p0 + GPg) * 128])
                w1_sb.append(t)
            w2_sb = wpool.tile([128, GPg * H], fwk, tag="w2")
            nc.sync.dma_start(out=w2_sb, in_=w2_d[:, p0 * H : (p0 + GPg) * H])

            # Layer 1: z1[h, col] per slot; even slot -> psum rows 0:64,
            # odd slot -> rows 64:128 (concurrent column-tiles of the PE).
            # Full-bank psum tiles (512 f32 = 2KB/partition): quarter-bank
            # tiles get packed into shared banks, and the bank-overlap
            # tracker then serializes cross-group matmuls with extra waits.
            ps1_full = p1pool.tile([128, 512], f32, tag="ps1")
            ps1 = ps1_full[:, :GC]
            # Per pair, emit k-inner/e-inner so consecutive matmuls alternate
            # PE column halves (LDWEIGHTS overlap) while each psum region's
            # k0/k1 accumulation pair stays adjacent modulo one instruction.
            for pr in range(GPg):
                pc = slice(pr * CAP, (pr + 1) * CAP)
                for k in range(2):
                    for e in range(2):
                        s = (p0 + pr) * 2 + e
                        nc.tensor.matmul(
                            out=ps1[64 * e : 64 * (e + 1), pc],
                            lhsT=w1_sb[k][:, pr * 128 + 64 * e : pr * 128 + 64 * (e + 1)],
                            rhs=xs_sb[k][:, s * CAP : (s + 1) * CAP],
                            start=(k == 0),
                            stop=(k == 1),
                        )
            h1 = hpool.tile([128, GC], fwk, tag="h1")
            nc.scalar.activation(out=h1, in_=ps1, func=GELU)

            # Layer 2: disjoint PE quadrants for even/odd slots.
            ps2_full = p2pool.tile([128, 512], f32, tag="ps2")
            ps2 = ps2_full[:, :GC]
            # PE anchor: claim the recycled ps2 bank with a whole-region
            # accumulate of +0 (rhs is a resident zero block; the real L2
            # matmuls below overwrite it with start=True). Its single PE
            # self-wait covers the bank's WAW against group g-2's last L2
            # matmul; without it that wait lands on the leading L2 matmul,
            # which then carries 3 sync waits and fails walrus codegen.
            nc.tensor.matmul(
                out=ps2,
                lhsT=w1_sb[1][:, 0:128],
                rhs=ones_sb[:, 2 : 2 + GC],
                start=False,
                stop=True,
                skip_group_check=True,
            )
            for pr in range(GPg):
                pc = slice(pr * CAP, (pr + 1) * CAP)
                for e in range(2):
                    rows = slice(64 * e, 64 * (e + 1))
                    nc.tensor.matmul(
                        out=ps2[rows, pc],
                        lhsT=w2_sb[rows, pr * H : (pr + 1) * H],
                        rhs=h1[rows, pc],
                        start=True,
                        stop=True,
                    )
            h2 = hpool.tile([128, GC], fwk, tag="h2")
            nc.scalar.activation(out=h2, in_=ps2, func=GELU)

            # Layer 3: elementwise h2 * l3, then per-half partition reduction
            # via a single matmul against the indicator columns.
            m = hpool.tile([128, GC], fwk, tag="m")
            nc.vector.tensor_mul(m, h2, w3_sb[:, c0 : c0 + GC])
            ps3_full = p3pool.tile([2, 512], f32, tag="ps3")
            ps3 = ps3_full[:, :GC]
            nc.tensor.matmul(out=ps3, lhsT=ones_sb[:, 0:2], rhs=m, start=True, stop=True)
            nc.vector.tensor_copy(logits_sb[:, c0 : c0 + GC], ps3)

            if g == NG - 1:
                nc.scalar.activation(out=outsb, in_=logits_sb, func=SIGM)
                nc.sync.dma_start(out=out_d[:], in_=outsb)

    # Bacc lowering: moves extra matmul waits onto LDWEIGHTS and splits
    # multi-wait instructions into event-semaphore prefixes (TRN2 allows at
    # most one sync wait per instruction).
    nc.compile()
    return nc


def _route(tids):
    """Group sample indices by task, pack into CAP-row slots per core.

    Returns (n_pairs, slot_task [NCORES, S], slot_sample [NCORES, S, CAP]).
    slot_sample is -1 where padded; slot_task is 0 for unused slots.
    """
    order = np.argsort(tids, kind="stable")
    counts = np.bincount(tids, minlength=NT)
    starts = np.zeros(NT + 1, dtype=np.int64)
    np.cumsum(counts, out=starts[1:])

    per_core = []
    for c in range(NCORES):
        slots = []  # (task, start_in_order, n)
        for t in range(c * TPC, (c + 1) * TPC):
            ct = int(counts[t])
            off = int(starts[t])
            while ct > 0:
                n = min(ct, CAP)
                slots.append((t, off, n))
                off += n
                ct -= n
        per_core.append(slots)

    s_needed = max(len(s) for s in per_core)
    S = max(4, ((s_needed + 1) // 2) * 2)
    n_pairs = S // 2

    slot_task = np.zeros((NCORES, S), dtype=np.int64)
    slot_sample = np.full((NCORES, S, CAP), -1, dtype=np.int64)
    for c in range(NCORES):
        for i, (t, off, n) in enumerate(per_core[c]):
            slot_task[c, i] = t
            slot_sample[c, i, :n] = order[off : off + n]
    return n_pairs, slot_task, slot_sample


def kernel(x, task_ids, l1_emb, l2_emb, l3_emb):
    if USE_BF16:
        import ml_dtypes

        fwk_np = ml_dtypes.bfloat16
    else:
        fwk_np = np.float32

    # Cast once up front: everything below is gather/transpose only, so the
    # result is bit-identical to casting at the end, at half the host traffic.
    x = np.asarray(x, dtype=np.float32).astype(fwk_np)
    tids = np.asarray(task_ids).astype(np.int64)
    l1 = np.asarray(l1_emb, dtype=np.float32).astype(fwk_np)
    l2 = np.asarray(l2_emb, dtype=np.float32).astype(fwk_np)
    l3 = np.asarray(l3_emb, dtype=np.float32).astype(fwk_np)
    B = x.shape[0]

    n_pairs, slot_task, slot_sample = _route(tids)
    S = 2 * n_pairs
    COLS = n_pairs * CAP

    ones2 = np.zeros((128, 2 + GCOLS), dtype=fwk_np)
    ones2[:64, 0] = 1.0
    ones2[64:, 1] = 1.0

    in_maps = []
    for c in range(NCORES):
        st = slot_task[c]
        ss = slot_sample[c]
        valid = ss >= 0

        # xs[k, p, s*CAP+j] = x[sample(s,j), 128*k+p]  (0 when padded)
        xg = x[np.where(valid, ss, 0).ravel()]
        xg[~valid.ravel()] = 0.0
        xs = np.ascontiguousarray(xg.T.reshape(2, 128, S * CAP))

        # w1[k, p, pr*128 + e*64 + h] = W1[slot 2pr+e][128k+p, h]
        w1_all = l1[st].reshape(S, F, H)
        w1 = np.ascontiguousarray(
            w1_all.reshape(n_pairs, 2, F, H).transpose(2, 0, 1, 3).reshape(F, n_pairs * 128)
        ).reshape(2, 128, n_pairs * 128)

        # w2[e*64+i, pr*64+j] = W2[slot 2pr+e][i, j]
        w2_all = l2[st].reshape(S, H, H)
        w2 = np.ascontiguousarray(
            w2_all.reshape(n_pairs, 2, H, H).transpose(1, 2, 0, 3).reshape(128, n_pairs * H)
        )

        # w3e[e*64+h, pr*CAP+j] = l3[slot 2pr+e][h]
        w3_all = l3[st].reshape(n_pairs, 2, H).transpose(1, 2, 0)  # [2, H, n_pairs]
        w3e = np.ascontiguousarray(
            np.broadcast_to(w3_all[:, :, :, None], (2, H, n_pairs, CAP)).reshape(128, COLS)
        )

        in_maps.append({"xs": xs, "w1": w1, "w2": w2, "w3e": w3e, "ones2": ones2})

    if n_pairs not in _PROGRAM_CACHE:
        _PROGRAM_CACHE[n_pairs] = _build_program(n_pairs)
    nc = _PROGRAM_CACHE[n_pairs]

    from concourse.bass_utils import run_bass_kernel_spmd

    global LAST_IN_MAPS, LAST_NPAIRS
    LAST_IN_MAPS, LAST_NPAIRS = in_maps, n_pairs
    res = run_bass_kernel_spmd(nc, in_maps, list(range(NCORES)))

    y = np.zeros(B, dtype=np.float32)
    e_idx = (np.arange(S) % 2)[:, None] * np.ones((1, CAP), dtype=np.int64)
    col_idx = (np.arange(S) // 2)[:, None] * CAP + np.arange(CAP)[None, :]
    for c in range(NCORES):
        out_c = res.results[c]["out"]  # [2, COLS]
        valid = slot_sample[c] >= 0
        y[slot_sample[c][valid]] = out_c[
            e_idx[valid].astype(np.int64), col_idx[valid].astype(np.int64)
        ]
    return y[:, None]


def measure_hw_ns(in_maps, n_pairs, passes=65, base_passes=17):
    """Estimate steady-state HW time per kernel execution.

    Builds a timing variant whose Bass program repeats the full group loop
    `passes` times over the same inputs (one PJRT custom call), and
    differences it against the single-pass program: (T_P - T_1)/(P - 1).
    The multi-ms axon dispatch overhead cancels in the difference.
    """
    import time

    import jax
    from jax.experimental.shard_map import shard_map
    from jax.sharding import Mesh, NamedSharding, PartitionSpec

    import concourse.mybir as mybir
    from concourse.bass2jax import _bass_exec_p, partition_id_tensor

    def runner(nc):
        partition_name = nc.partition_id_tensor.name if nc.partition_id_tensor else None
        in_names, out_names, out_avals = [], [], []
        for alloc in nc.m.functions[0].allocations:
            if not isinstance(alloc, mybir.MemoryLocationSet):
                continue
            name = alloc.memorylocations[0].name
            if alloc.kind == "ExternalInput":
                if name != partition_name:
                    in_names.append(name)
            elif alloc.kind == "ExternalOutput":
                out_names.append(name)
                out_avals.append(
                    jax.core.ShapedArray(
                        tuple(alloc.tensor_shape), mybir.dt.np(alloc.dtype)
                    )
                )
        n_params = len(in_names)
        in_names_all = in_names + out_names + ([partition_name] if partition_name else [])

        def _body(*args):
            operands = list(args)
            if partition_name is not None:
                operands.append(partition_id_tensor())
            return tuple(
                _bass_exec_p.bind(
                    *operands,
                    out_avals=tuple(out_avals),
                    in_names=tuple(in_names_all),
                    out_names=tuple(out_names),
                    lowering_input_output_aliases=(),
                    sim_require_finite=True,
                    sim_require_nnan=True,
                    nc=nc,
                )
            )

        devices = jax.devices()[:NCORES]
        mesh = Mesh(np.asarray(devices), ("core",))
        specs_in = (PartitionSpec("core"),) * (n_params + len(out_names))
        specs_out = (PartitionSpec("core"),) * len(out_names)
        fn = jax.jit(
            shard_map(
                _body, mesh=mesh, in_specs=specs_in, out_specs=specs_out, check_rep=False
            ),
            keep_unused=True,
        )
        sh = NamedSharding(mesh, PartitionSpec("core"))
        args = [
            jax.device_put(
                np.concatenate([np.asarray(m[name]) for m in in_maps], axis=0), sh
            )
            for name in in_names
        ]
        for av in out_avals:
            args.append(
                jax.device_put(
                    np.zeros((NCORES * av.shape[0], *av.shape[1:]), av.dtype), sh
                )
            )
        return fn, args

    for p in (base_passes, passes):
        if (n_pairs, p) not in _PROGRAM_CACHE:
            _PROGRAM_CACHE[(n_pairs, p)] = _build_program(n_pairs, passes=p)

    fn1, args1 = runner(_PROGRAM_CACHE[(n_pairs, base_passes)])
    fnP, argsP = runner(_PROGRAM_CACHE[(n_pairs, passes)])
    jax.block_until_ready(fn1(*args1))
    jax.block_until_ready(fnP(*argsP))

    def batch(fn, args, k=50):
        t0 = time.perf_counter()
        out = None
        for _ in range(k):
            out = fn(*args)
        jax.block_until_ready(out)
        return time.perf_counter() - t0

    # Pipelined batches: blocking single calls quantize at the axon
    # completion-poll interval (~100 ms), so difference K unblocked calls.
    k = 40
    estimates = []
    for _ in range(5):
        t1 = batch(fn1, args1, k)
        tp = batch(fnP, argsP, k)
        estimates.append((tp - t1) / (k * (passes - base_passes)) * 1e9)
    estimates.sort()
    return estimates[len(estimates) // 2]

